# revision 35
# baseline (speedup 1.0000x reference)
"""Trainium2 Bass kernel for nn_MAEGIN (GIN message passing, 5 layers + decoder).

Strategy (8 NeuronCores, one chip):
- Nodes sharded contiguously: core c owns rows [c*6250, (c+1)*6250).
- Node features live on-chip feature-major only (hT [2][128 feat, 6272
  nodes], bf16); node-major copies exist transiently in small staging tiles
  that feed the AllGather input buffers.  fp32 PSUM accumulation everywhere.
- Each GIN aggregation: the node-feature table is AllGathered in TWO halves
  (local chunks 0-19 -> ag_out0 [20480, 256], chunks 20-48 -> ag_out1
  [29696, 256]); both halves are int16-index safe, and each collective is
  triggered as soon as its staged inputs exist so it overlaps gather
  emission of the surrounding layers.  Edges are grouped per (source-half,
  dst-chunk); one dma_gather per group (~1100 idxs, 16-granular counts)
  keeps each call under the ~2k-descriptor SWDGE ring, and strict rotation
  across the 4 SWDGE queues lets the Q7 issue at ~1.3ns/idx while four
  queue drains (~60 GB/s each) proceed in parallel; an 8-deep gather-tile
  pool decouples issue from drain+consume latency.  This matters because Q7
  descriptor emission + per-queue drain is the critical resource (~5ns/edge
  serialized if mismanaged).  One-hot selection matrices built on DVE
  (is_equal vs iota); PE matmuls accumulate per-dst-chunk segment sums in
  PSUM.  Pass A (h0 sources) stages partials in bf16; pass B adds h1
  sources, the self term (transpose-matmul from hT), and the staged partial
  (identity matmul), so no DVE copies touch the path (DVE copy/cast 2-port
  perf mode would block GPSIMD SWDGE descriptor writes).  All PSUM->SBUF
  copies run on ScalarE.
- The next layer's dense stage (W1/BN+PReLU/W2/BN+PReLU/residual, fused on
  ScalarE) is interleaved tile-by-tile into pass B via a callback, so dense
  PE work, ag_in staging DMA and the AllGather all overlap the remaining
  aggregation.
- Decoder folded on host: agg(h@Wp.T)@Wt.T == agg(h)@(Wt@Wp).T, so the
  decoder is one aggregation + one combined matmul + the [nodes x 4096]
  logits matmul, all interleaved into the decoder aggregation the same way;
  vocab bias added on DVE; output rows written alternately on the two HWDGE
  rings (sync/scalar).
"""
import math
import numpy as np
import ml_dtypes

import concourse.bass as bass
import concourse.bacc as bacc
import concourse.tile as tile
import concourse.mybir as mybir
from concourse.bass_utils import run_bass_kernel_spmd

P = 128
D = 256
NCORES = 8
N_NODES = 50000
N_EDGES = 800000
VOCAB = 4096
L = 5
BN_EPS = 1e-5

NL = N_NODES // NCORES          # 6250 nodes per core
NCHUNK = math.ceil(NL / P)      # 49 dst chunks
NLP = NCHUNK * P                # 6272 padded nodes per core
LAST_VALID = NL - (NCHUNK - 1) * P  # 106 valid rows in last chunk
H0C = 20                        # local chunks in ag half 0 (multiple of 4)
H1C = NCHUNK - H0C              # 25 chunks in ag half 1
H0N = H0C * P                   # 3072 rows per core in half 0
H1N = H1C * P                   # 3200 rows per core in half 1
AG0 = H0N * NCORES              # 24576 rows in ag_out0 (int16-safe)
AG1 = H1N * NCORES              # 25600 rows in ag_out1 (int16-safe)
# one gather call per (src-half, dst-chunk): ~1100 idxs/call keeps each call
# under the ~2048-descriptor SWDGE ring so emission never stalls on drain;
# queue rotation then overlaps the four rings' drains.
NT_SIZES = [512] * (NLP // 512) + ([NLP % 512] if NLP % 512 else [])  # 12x512+128
NT_H0 = H0N // 512              # 6 n-tiles cover dense half 0 (3072 cols)

f32 = mybir.dt.float32
f16 = mybir.dt.float16
bf16 = mybir.dt.bfloat16
i16 = mybir.dt.int16

PAD_DOFF = 200.0  # dst offset for padding edges: matches no iota value -> zero row


def _wrap16(idx: np.ndarray) -> np.ndarray:
    """[L] int array -> [128, L//16] wrapped-16 int16, replicated across q7 groups."""
    Ln = len(idx)
    assert Ln % 16 == 0
    base = idx.reshape(Ln // 16, 16).T.astype(np.int16)
    return np.ascontiguousarray(np.tile(base, (8, 1)))


def _prepare(x, edge_index, emb, W1s, b1s, g1s, be1s, m1s, v1s, a1s,
             W2s, b2s, g2s, be2s, m2s, v2s, a2s, Wproj, Wtrn, btrn, Wprd, bprd):
    """Host-side sharding/preprocessing. Returns (schedule, in_maps)."""
    x = np.asarray(x).astype(np.int64)
    src = np.asarray(edge_index[0]).astype(np.int64)
    dst = np.asarray(edge_index[1]).astype(np.int64)

    # ag table row for every edge source: h_nm[p, c, :] -> half row p*HC + c
    # within the core block; local node j = c*128 + p.
    s_core = src // NL
    loc = src % NL
    lp = loc % P
    lc = loc // P
    half = (lc >= H0C).astype(np.int64)
    row_h0 = s_core * H0N + lp * H0C + lc
    row_h1 = s_core * H1N + lp * H1C + (lc - H0C)
    ag_row = np.where(half == 0, row_h0, row_h1)
    assert ag_row.min() >= 0
    assert ag_row[half == 0].max() < AG0 and ag_row[half == 1].max() < AG1

    dst_core = dst // NL
    dst_loc = dst % NL
    chunk = dst_loc // P
    doff = dst_loc % P

    # group edges per core by (src half, dst chunk)  [pass-major ordering]
    counts = np.zeros((NCORES, 2, NCHUNK), dtype=np.int64)
    per_core = []
    for c in range(NCORES):
        m = dst_core == c
        key = half[m] * NCHUNK + chunk[m]
        order = np.argsort(key, kind="stable")
        cnt = np.bincount(key, minlength=2 * NCHUNK).reshape(2, NCHUNK)
        counts[c] = cnt
        per_core.append((ag_row[m][order], doff[m][order], cnt))

    maxcnt = counts.max(axis=0)                      # [2, 49]
    tiles = np.maximum(np.ceil(maxcnt / P).astype(np.int64), 1)
    cnt16 = np.maximum(np.ceil(maxcnt / 16).astype(np.int64) * 16, 16)
    TT = int(tiles.sum())
    EIDX = int(cnt16.sum())

    flat_tiles = tiles.reshape(-1)
    tile_start = np.concatenate([[0], np.cumsum(flat_tiles)])[:-1].reshape(2, NCHUNK)
    idx_start = np.concatenate(
        [[0], np.cumsum(cnt16.reshape(-1))])[:-1].reshape(2, NCHUNK)

    eidx_maps, doff_maps = [], []
    for c in range(NCORES):
        rows_s, doff_s, cnt = per_core[c]
        grp_start = np.concatenate([[0], np.cumsum(cnt.reshape(-1))])
        idx_pad = np.zeros(EIDX, dtype=np.int64)
        doff_pad = np.full(TT * P, int(PAD_DOFF), dtype=np.int64)
        for h in range(2):
            for p in range(NCHUNK):
                g = h * NCHUNK + p
                n = int(cnt[h, p])
                if n == 0:
                    continue
                idx_pad[idx_start[h, p]:idx_start[h, p] + n] = \
                    rows_s[grp_start[g]:grp_start[g] + n]
                s = int(tile_start[h, p]) * P
                doff_pad[s:s + n] = doff_s[grp_start[g]:grp_start[g] + n]
        assert idx_pad.min() >= 0 and idx_pad.max() <= 32767
        eidx_maps.append(_wrap16(idx_pad))
        doff_maps.append(np.ascontiguousarray(
            doff_pad.reshape(TT, P).T.astype(ml_dtypes.bfloat16)))

    # embedding gather indices (per core, padded to NLP with 0)
    xw_maps = []
    for c in range(NCORES):
        xp = np.zeros(NLP, dtype=np.int64)
        xp[:NL] = x[c * NL:(c + 1) * NL]
        xw_maps.append(_wrap16(xp))

    # weights, shared across cores
    def lhst(W):  # W [out, in] -> [128, ki, mo, 128] with slice = W.T chunk
        o, i = W.shape
        ko, mo = i // P, o // P
        return np.ascontiguousarray(
            W.T.reshape(ko, P, mo, P).transpose(1, 0, 2, 3).astype(ml_dtypes.bfloat16))

    w1t = np.stack([lhst(W1s[l]) for l in range(L)])  # [L,128,2,2,128]
    w1t = np.ascontiguousarray(w1t.transpose(1, 0, 2, 3, 4))  # [128,L,2,2,128]
    w2t = np.stack([lhst(W2s[l]) for l in range(L)])
    w2t = np.ascontiguousarray(w2t.transpose(1, 0, 2, 3, 4))
    # decoder fold: agg(h@Wp.T)@Wt.T == agg(h) @ (Wt@Wp).T
    wct = lhst(np.asarray(Wtrn, np.float64) @ np.asarray(Wproj, np.float64))
    wprdt = np.ascontiguousarray(
        Wprd.T.reshape(2, P, VOCAB).transpose(1, 0, 2).astype(ml_dtypes.bfloat16))

    # folded BN scale/shift: per (l, sub, half) -> col l*4+sub*2+half
    bnsc = np.zeros((P, L * 4), dtype=np.float32)
    bnsh = np.zeros((P, L * 4), dtype=np.float32)
    alph = np.zeros((P, L * 2), dtype=np.float32)
    for l in range(L):
        for sub, (g, be, m, v, a) in enumerate(
                [(g1s[l], be1s[l], m1s[l], v1s[l], a1s[l]),
                 (g2s[l], be2s[l], m2s[l], v2s[l], a2s[l])]):
            sc = (g / np.sqrt(v + BN_EPS)).astype(np.float32)
            sh = (be - m * sc).astype(np.float32)
            for mo in range(2):
                bnsc[:, l * 4 + sub * 2 + mo] = sc[mo * P:(mo + 1) * P]
                bnsh[:, l * 4 + sub * 2 + mo] = sh[mo * P:(mo + 1) * P]
            alph[:, l * 2 + sub] = float(np.asarray(a))
    btrn2 = np.ascontiguousarray(btrn.reshape(2, P).T.astype(np.float32))
    bprdb = np.ascontiguousarray(
        np.tile(bprd, (P, 1)).astype(ml_dtypes.bfloat16))

    iota = np.tile(np.arange(P, dtype=np.float32), (P, 1)).astype(ml_dtypes.bfloat16)
    identb = np.eye(P, dtype=np.float32).astype(ml_dtypes.bfloat16)
    embt = np.asarray(emb).astype(ml_dtypes.bfloat16)

    def flat(a):
        return a.reshape(P, -1)
    bf_shared = [flat(iota), flat(identb), flat(bprdb), flat(w1t), flat(w2t),
                 flat(wct), flat(wprdt)]
    offs = {}
    o = 0
    for name, a in zip(["iota", "identb", "bprd", "w1", "w2", "wc",
                        "wprd"], bf_shared):
        offs[name] = o
        o += a.shape[1]
    offs["doff"] = o
    bfw = o + TT
    f32_shared = np.concatenate([bnsc, bnsh, alph, btrn2], axis=1).astype(np.float32)
    offs["bnsc"], offs["bnsh"], offs["alph"], offs["btrn"] = (
        0, L * 4, L * 8, L * 10)
    f32w = f32_shared.shape[1]
    i16w = EIDX // 16 + NLP // 16  # eidx then xw

    in_maps = []
    for c in range(NCORES):
        bfp = np.concatenate(bf_shared + [doff_maps[c]], axis=1)
        i16p = np.concatenate([eidx_maps[c], xw_maps[c]], axis=1)
        # per-core 16-rounded gather counts (runtime register values)
        c16 = np.maximum(
            np.ceil(counts[c] / 16).astype(np.int64) * 16, 16).reshape(-1)
        cnts = np.tile(c16.astype(np.int32), (P, 1))
        in_maps.append(dict(
            bfp=np.ascontiguousarray(bfp.astype(ml_dtypes.bfloat16)),
            f32p=np.ascontiguousarray(f32_shared),
            i16p=np.ascontiguousarray(i16p.astype(np.int16)),
            cnts=np.ascontiguousarray(cnts),
            embt=embt))

    sched = dict(tiles=tiles, tile_start=tile_start, TT=TT, EIDX=EIDX,
                 cnt16=cnt16, idx_start=idx_start,
                 offs=offs, bfw=bfw, f32w=f32w, i16w=i16w)
    return sched, in_maps


def _build(sched):
    tiles = sched["tiles"]          # [2, 49]
    tile_start = sched["tile_start"]
    cnt16 = sched["cnt16"]
    idx_start = sched["idx_start"]
    EIDX = sched["EIDX"]
    offs = sched["offs"]
    TMAXC = int(tiles.max())
    assert TMAXC >= 10, TMAXC

    nc = bacc.Bacc("TRN2", target_bir_lowering=False, num_swdge_queues=4)

    bfp = nc.dram_tensor("bfp", [P, sched["bfw"]], bf16, kind="ExternalInput")
    f32p = nc.dram_tensor("f32p", [P, sched["f32w"]], f32, kind="ExternalInput")
    i16p = nc.dram_tensor("i16p", [P, sched["i16w"]], i16, kind="ExternalInput")
    cnts = nc.dram_tensor("cnts", [P, 2 * NCHUNK], mybir.dt.int32,
                          kind="ExternalInput")
    embt = nc.dram_tensor("embt", [VOCAB, D], bf16, kind="ExternalInput")

    out = nc.dram_tensor("out", [NL, VOCAB], f32, kind="ExternalOutput")

    ag_in0 = nc.dram_tensor("ag_in0", [H0N, D], bf16)
    ag_in1 = nc.dram_tensor("ag_in1", [H1N, D], bf16)
    ag_out0 = nc.dram_tensor("ag_out0", [AG0, D], bf16, addr_space="Shared")
    ag_out1 = nc.dram_tensor("ag_out1", [AG1, D], bf16, addr_space="Shared")

    with tile.TileContext(nc) as tc:
        with (
            tc.tile_pool(name="cst", bufs=1) as cst,
            tc.tile_pool(name="gp", bufs=8) as gp,
            tc.tile_pool(name="stp", bufs=2) as stp,
            tc.tile_pool(name="selp", bufs=5) as selp,
            tc.tile_pool(name="asbp", bufs=3) as asbp,
            tc.tile_pool(name="op", bufs=3) as op,
            tc.tile_pool(name="accp", bufs=3, space="PSUM") as accp,
            tc.tile_pool(name="trp", bufs=2, space="PSUM") as trp,
            tc.tile_pool(name="mmp", bufs=3, space="PSUM") as mmp,
        ):
            bf_sb = cst.tile([P, sched["bfw"]], bf16, tag="bf")
            nc.sync.dma_start(bf_sb[:], bfp[:])
            f_sb = cst.tile([P, sched["f32w"]], f32, tag="f32")
            nc.sync.dma_start(f_sb[:], f32p[:])
            i_sb = cst.tile([P, sched["i16w"]], i16, tag="i16")
            nc.sync.dma_start(i_sb[:], i16p[:])

            def bfs(name, j0, w):
                s = offs[name] + j0
                return bf_sb[:, s:s + w]

            iota_sb = bfs("iota", 0, P)
            identb_sb = bfs("identb", 0, P)

            def w_ap(name, l, ki, mo):
                return bfs(name, ((l * 2 + ki) * 2 + mo) * P, P)

            def bn_ap(kind, i):
                s = offs[kind] + i
                return f_sb[:, s:s + 1]

            hT = [cst.tile([P, NLP], bf16, tag=f"hT{f}", name=f"hT{f}")
                  for f in range(2)]
            aggT = [cst.tile([P, NLP], bf16, tag=f"aggT{f}", name=f"aggT{f}")
                    for f in range(2)]
            aggA = cst.tile([P, NCHUNK, D], bf16, tag="aggA")   # pass-A staging

            qctr = [0]

            def nextq():
                qctr[0] = (qctr[0] + 1) % 4
                return qctr[0]

            hwctr = [0]

            def hw_dma(dst, src_ap):
                hwctr[0] ^= 1
                (nc.sync if hwctr[0] else nc.scalar).dma_start(dst, src_ap)

            def act_copy(dst, src_ap):
                nc.scalar.activation(
                    out=dst, in_=src_ap,
                    func=mybir.ActivationFunctionType.Copy)

            # zero the gather pool once: slots beyond a call's index count
            # keep stale data (sel zeroes their contribution, but the first
            # use must not contain NaN/Inf bit patterns)
            for _ in range(8):
                gtz = gp.tile([P, TMAXC, D], bf16, tag="g")
                nc.vector.memset(gtz[:], 0.0)

            # ---- embedding gather -> hT and ag_in (via gather-pool tiles) ----
            xw0 = EIDX // 16

            ag_in_v = [ag_in0[:].rearrange("(p c) f -> p c f", p=P),
                       ag_in1[:].rearrange("(p c) f -> p c f", p=P)]

            def emb_group(c0, nch):
                gt = gp.tile([P, TMAXC, D], bf16, tag="g")
                nc.gpsimd.dma_gather(
                    gt[:, :nch, :], embt[:],
                    i_sb[:, xw0 + c0 * 8:xw0 + (c0 + nch) * 8],
                    nch * P, nch * P, D,
                    single_packet=False, queue_num=nextq())
                for i in range(nch):
                    p = c0 + i
                    for fh in range(2):
                        tp = trp.tile([P, P], bf16, space="PSUM", tag="trb")
                        nc.tensor.transpose(
                            tp[:], gt[:, i, fh * P:(fh + 1) * P], identb_sb)
                        act_copy(hT[fh][:, p * P:(p + 1) * P], tp[:])
                if p < H0C:
                    nc.sync.dma_start(ag_in_v[0][:, c0:c0 + nch, :],
                                      gt[:, :nch, :])
                else:
                    nc.sync.dma_start(ag_in_v[1][:, c0 - H0C:c0 - H0C + nch, :],
                                      gt[:, :nch, :])

            def ag_trig(halfv):
                if halfv == 0:
                    nc.gpsimd.collective_compute(
                        "AllGather", mybir.AluOpType.bypass,
                        replica_groups=[list(range(NCORES))],
                        ins=[ag_in0[:]], outs=[ag_out0[:]])
                else:
                    nc.gpsimd.collective_compute(
                        "AllGather", mybir.AluOpType.bypass,
                        replica_groups=[list(range(NCORES))],
                        ins=[ag_in1[:]], outs=[ag_out1[:]])

            def do_agg(dst_fm, dense_cb=None):
                """GIN aggregate: dst_fm[f][feat,node] = sum_{j->i} h[j] + h[i].

                Pass A gathers h0 sources for all dst chunks (staged to aggA);
                the AG of table half 1 is triggered after pass-A chunk 5 so it
                overlaps the remaining pass-A emission.  Pass B adds h1
                sources, the self term, and the staged partial, then
                transposes into dst_fm.  dense_cb(k), if given, is invoked
                after pass-B finishes the chunks covered by dense n-tile k, so
                the next layer's dense work and AG publication interleave with
                the remaining aggregation.
                """
                for h in range(2):
                    table = ag_out0 if h == 0 else ag_out1
                    for p in range(NCHUNK):
                        Tp = int(tiles[h, p])
                        n16 = int(cnt16[h, p])
                        gt = gp.tile([P, TMAXC, D], bf16, tag="g")
                        c0 = int(idx_start[h, p]) // 16
                        nc.gpsimd.dma_gather(
                            gt[:, :Tp, :], table[:],
                            i_sb[:, c0:c0 + n16 // 16],
                            n16, n16, D, single_packet=(n16 <= 1024),
                            queue_num=nextq())
                        if h == 0 and p == 5:
                            # table half 1 collective runs under pass-A
                            ag_trig(1)
                        sel = selp.tile([P, TMAXC, P], bf16, tag="sel")
                        ts = int(tile_start[h, p])
                        dslc = bf_sb[:, offs["doff"] + ts:offs["doff"] + ts + Tp]
                        nc.vector.tensor_tensor(
                            out=sel[:, :Tp, :],
                            in0=dslc[:, :, None].to_broadcast([P, Tp, P]),
                            in1=iota_sb[:, None, :].to_broadcast([P, Tp, P]),
                            op=mybir.AluOpType.is_equal,
                        )
                        acc = accp.tile([P, D], f32, space="PSUM", tag="acc")
                        for t in range(Tp):
                            nc.tensor.matmul(
                                out=acc[:], lhsT=sel[:, t, :],
                                rhs=gt[:, t, :],
                                start=(t == 0),
                                stop=(h == 0 and t == Tp - 1))
                        if h == 0:
                            act_copy(aggA[:, p, :], acc[:])
                        else:
                            for fh in range(2):
                                nc.tensor.matmul(
                                    out=acc[:, fh * P:(fh + 1) * P],
                                    lhsT=hT[fh][:, p * P:(p + 1) * P],
                                    rhs=identb_sb,
                                    start=False, stop=False)
                            nc.tensor.matmul(
                                out=acc[:], lhsT=identb_sb,
                                rhs=aggA[:, p, :],
                                start=False, stop=True)
                            asb = asbp.tile([P, D], bf16, tag="asb")
                            act_copy(asb[:], acc[:])
                            for fh in range(2):
                                tp = trp.tile([P, P], bf16, space="PSUM",
                                              tag="trb")
                                nc.tensor.transpose(
                                    tp[:], asb[:, fh * P:(fh + 1) * P],
                                    identb_sb)
                                act_copy(
                                    dst_fm[fh][:, p * P:(p + 1) * P],
                                    tp[:])
                            if dense_cb is not None and (
                                    p % 4 == 3 or p == NCHUNK - 1):
                                dense_cb(p // 4)

            def nsl(n):
                s = sum(NT_SIZES[:n])
                return slice(s, s + NT_SIZES[n])

            def dense_tiles(l, nlist, src_fm):
                """W1/BN/PReLU/W2/BN/PReLU/residual for the given n-tiles."""
                for n in nlist:
                    w = NT_SIZES[n]
                    o1s = op.tile([P, 2, 512], bf16, tag="o1s")
                    for mo in range(2):
                        pm = mmp.tile([P, 512], f32, space="PSUM", tag="pm")
                        nc.tensor.matmul(pm[:, :w], w_ap("w1", l, 0, mo),
                                         src_fm[0][:, nsl(n)], start=True, stop=False)
                        nc.tensor.matmul(pm[:, :w], w_ap("w1", l, 1, mo),
                                         src_fm[1][:, nsl(n)], start=False, stop=True)
                        i = l * 4 + 0 * 2 + mo
                        nc.scalar.activation(
                            out=o1s[:, mo, :w], in_=pm[:, :w],
                            func=mybir.ActivationFunctionType.Prelu,
                            bias=bn_ap("bnsh", i), scale=bn_ap("bnsc", i),
                            alpha=bn_ap("alph", l * 2))
                    for mo in range(2):
                        pm = mmp.tile([P, 512], f32, space="PSUM", tag="pm")
                        nc.tensor.matmul(pm[:, :w], w_ap("w2", l, 0, mo),
                                         o1s[:, 0, :w], start=True, stop=False)
                        nc.tensor.matmul(pm[:, :w], w_ap("w2", l, 1, mo),
                                         o1s[:, 1, :w], start=False, stop=True)
                        i = l * 4 + 1 * 2 + mo
                        o2 = op.tile([P, 512], bf16, tag="o2")
                        nc.scalar.activation(
                            out=o2[:, :w], in_=pm[:, :w],
                            func=mybir.ActivationFunctionType.Prelu,
                            bias=bn_ap("bnsh", i), scale=bn_ap("bnsc", i),
                            alpha=bn_ap("alph", l * 2 + 1))
                        nc.vector.tensor_tensor(
                            out=hT[mo][:, nsl(n)], in0=hT[mo][:, nsl(n)],
                            in1=o2[:, :w], op=mybir.AluOpType.add)

            # ---- pipeline: publish h tables, aggregate, dense ----
            def make_dense_cb(l):
                def dense_cb(k):
                    dense_tiles(l, [k], aggT)
                    chunks = list(range(4 * k, min(4 * k + 4, NCHUNK)))
                    st = stp.tile([P, 4, D], bf16, tag="st")
                    for i, p in enumerate(chunks):
                        for fh in range(2):
                            tp = trp.tile([P, P], bf16, space="PSUM",
                                          tag="trb")
                            nc.tensor.transpose(
                                tp[:], hT[fh][:, p * P:(p + 1) * P],
                                identb_sb)
                            act_copy(st[:, i, fh * P:(fh + 1) * P], tp[:])
                    nch = len(chunks)
                    c0 = chunks[0]
                    if c0 < H0C:
                        nc.sync.dma_start(ag_in_v[0][:, c0:c0 + nch, :],
                                          st[:, :nch, :])
                    else:
                        nc.sync.dma_start(
                            ag_in_v[1][:, c0 - H0C:c0 - H0C + nch, :],
                            st[:, :nch, :])
                    if k == NT_H0 - 1:
                        ag_trig(0)
                return dense_cb

            for c0, nch in [(0, 10), (10, 10), (20, 10), (30, 10), (40, 9)]:
                emb_group(c0, nch)
                if c0 + nch == H0C:
                    ag_trig(0)
            for l in range(L):
                do_agg(aggT, dense_cb=make_dense_cb(l))
            # after layer L-1 this published h_L -> decoder aggregation input

            # ---- decoder: wc matmul + logits interleaved into the agg ----
            decT = [cst.tile([P, NLP], bf16, tag=f"hT{f}", name=f"decT{f}")
                    for f in range(2)]
            NV = VOCAB // 512

            def dec_cb(k):
                w = NT_SIZES[k]
                # combined (Wtrn@Wproj) matmul + btrn bias for tile k
                for mo in range(2):
                    pm = mmp.tile([P, 512], f32, space="PSUM", tag="pm")
                    nc.tensor.matmul(pm[:, :w], w_ap("wc", 0, 0, mo),
                                     aggT[0][:, nsl(k)], start=True, stop=False)
                    nc.tensor.matmul(pm[:, :w], w_ap("wc", 0, 1, mo),
                                     aggT[1][:, nsl(k)], start=False, stop=True)
                    nc.scalar.activation(
                        out=decT[mo][:, nsl(k)], in_=pm[:, :w],
                        func=mybir.ActivationFunctionType.Identity,
                        bias=bn_ap("btrn", mo), scale=1.0)
                # logits for the chunks covered by tile k
                for p in range(4 * k, min(4 * k + 4, NCHUNK)):
                    rows = P if p < NCHUNK - 1 else LAST_VALID
                    for v in range(NV):
                        vs = slice(v * 512, (v + 1) * 512)
                        pm = mmp.tile([P, 512], f32, space="PSUM", tag="pm")
                        nc.tensor.matmul(pm[:], decT[0][:, p * P:(p + 1) * P],
                                         bfs("wprd", 0 * VOCAB + v * 512, 512),
                                         start=True, stop=False)
                        nc.tensor.matmul(pm[:], decT[1][:, p * P:(p + 1) * P],
                                         bfs("wprd", 1 * VOCAB + v * 512, 512),
                                         start=False, stop=True)
                        ob = op.tile([P, 512], f32, tag="ob")
                        nc.vector.tensor_tensor(
                            out=ob[:], in0=pm[:], in1=bfs("bprd", v * 512, 512),
                            op=mybir.AluOpType.add)
                        hw_dma(out[p * P:p * P + rows, vs], ob[:rows, :])

            do_agg(aggT, dense_cb=dec_cb)

    nc.compile()
    return nc


def run_sharded(inputs: dict, trace: bool = False, trace_kwargs=None, tmpdir=None):
    sched, in_maps = _prepare(**inputs)
    nc = _build(sched)
    kw = {}
    if trace:
        kw = dict(trace=True, tmpdir=tmpdir)
        if trace_kwargs:
            kw["trace_kwargs"] = trace_kwargs
    res = run_bass_kernel_spmd(nc, in_maps, core_ids=list(range(NCORES)), **kw)
    full = np.concatenate([res.results[c]["out"] for c in range(NCORES)], axis=0)
    return full, res


def kernel(**inputs) -> np.ndarray:
    out, _ = run_sharded(inputs, trace=False)
    return out
